# revision 36
# baseline (speedup 1.0000x reference)
"""Embedding-lookup kernel for Trainium2 (Bass/Tile), 8-core data-parallel.

Problem: out[b, l] = prototypes[labels[b, l]]
  inputs     (512, 21, 1, 29, 129) f32  -- unused except for batch size
  labels     (512, 21) int64            -- values in [0, 25)
  prototypes (25, 1, 29, 129) f32
  out        (512, 21, 1, 29, 129) f32  (~161 MB)

Strategy (memory regime): shard the batch dim across 8 cores (1344 lookups
per core, 20.1 MB of output writes each). Per core the gather runs as
one-hot @ table matmuls on the PE, streaming PSUM -> SBUF -> DRAM so HBM
traffic is write-only. Default mode "v2": the f32 table is host-split into
three bf16 planes (hi/mid/lo at partitions 0/32/64, K=96) whose sum
reconstructs every f32 exactly; the one-hot is built on device from the
label vector (PE ones-broadcast + iota/is_equal) -- with 0/1 weights the
gather is bit-exact. Inputs load as two fat planes chunks on the SP HWDGE
ring plus the tiny label DMA on the ACT ring.

Schedule notes (from perfetto/NTFF traces on the axon trn2 pool):
 - the 16 SDMA engines cap at ~24.7 GB/s each (~420 GB/s/core steady);
   output DMAs keep 128 descriptors (1 per partition) because descriptors
   round-robin positionally over engines and odd counts skew the load
 - input DMAs use >=2KB per-partition descriptors and ride both HWDGE
   rings (planes on SP, one-hot on ACT); the first output DMA otherwise
   queues behind input packets in the ring FIFO
 - the PE comes up HAM-clock-gated at half rate; dummy matmuls on scratch
   SBUF while inputs are in flight pull the full-rate transition earlier
 - the first row tile streams per 512-col matmul (copy + DMA per half
   pair) to start the write stream ~4 us earlier
Measured (mode v2): 67.8-68.3 us HW exec in quiet windows, ~74-80 us when
the shared chip/HBM or DMA engine 15 is contended; bit-exact vs the f32
reference. Restructured variants (v3 host-built one-hot, v4 permuted
layout) measured slower and stay selectable for reference.
"""

import json

import numpy as np

import concourse.bass as bass
import concourse.mybir as mybir
from concourse.tile import TileContext
from concourse.bass_utils import run_bass_kernel_spmd

B, L, NCHAN, T, F = 512, 21, 1, 29, 129
D = NCHAN * T * F            # 3741 features per prototype
N_PROTO = 25
N_CORES = 8
B_PER_CORE = B // N_CORES    # 64
ROWS = B_PER_CORE * L        # 1344 lookups per core

ROW_TILE = 128               # output rows per matmul (PSUM partition dim)
COL_TILE = 512               # output cols per matmul (one PSUM bank of f32)

# "v2" (exact; host-split bf16 planes, on-device one-hot build) is the
# default: across today's A/B sessions it holds a 67.9-68.3 us band and
# rarely triggers the slow-DMA-engine mode, beating every restructuring
# attempt ("v3" host-built one-hot + tuned schedule: 69.1+ us and ~50%
# slow-mode; "v4" permuted row layout: worse). "k75"/"bf16x3" are
# on-device splits, "f32"/"f32r" probes only.
_MODE = "v2"

# v4 layout: partition p holds RUNS[r] consecutive output rows as
# contiguous slots. SBUF port 15 serves partitions 92-95/124-127; its DMA
# engine is intermittently ~20% slower, so those partitions get 8 slots
# while the rest get 10-11. (start_partition, n_partitions, slots)
RUNS = [
    (0, 32, 10),
    (32, 60, 11),
    (92, 4, 8),
    (96, 20, 11),
    (116, 8, 10),
    (124, 4, 8),
]
MAX_S = 11
assert sum(np_ * s for _, np_, s in RUNS) == ROWS


def _run_bases() -> list[int]:
    bases, acc = [], 0
    for _, np_, s in RUNS:
        bases.append(acc)
        acc += np_ * s
    return bases




GP = 32                  # partition stride between the three plane groups
KDIM = 3 * GP            # 96 = matmul contraction dim incl. zero pads
KDENSE = 3 * N_PROTO     # 75 = dense contraction dim (host-packed planes)
import os as _os
V3_DENSE = _os.environ.get("V3_DENSE", "1") == "1"
KD = KDENSE if V3_DENSE else KDIM


def _split_multiwaits(bir: dict) -> int:
    """This walrus build allows at most one sync-wait per instruction on
    several instruction encodings; Tile attaches one wait per dependency.
    Hoist every wait of a multi-wait instruction into its own EventSemaphore
    (the encoding `wait_ge` uses) inserted directly before it on the same
    engine. Returns the number of instructions split."""
    n_split = 0
    ctr = 0
    for f in bir["functions"]:
        for blk in f["blocks"]:
            insts = blk["instructions"]
            out = []
            for inst in insts:
                si = inst.get("sync_info")
                waits = (si or {}).get("on_wait") or []
                if len(waits) > 1:
                    n_split += 1
                    for w in waits:
                        ctr += 1
                        out.append(
                            {
                                "debug": inst.get("debug", 0),
                                "engine": inst["engine"],
                                "ins": [],
                                "outs": [],
                                "name": f"mwsplit-{ctr}",
                                "opcode": "EventSemaphore",
                                "sync_info": {"on_update": [], "on_wait": [w]},
                            }
                        )
                    si["on_wait"] = []
                out.append(inst)
            blk["instructions"] = out
    return n_split


def _install_multiwait_splitter(nc: bass.Bass) -> None:
    orig = nc.to_json_bytes

    def patched() -> bytes:
        bir = json.loads(orig())
        _split_multiwaits(bir)
        return json.dumps(bir).encode()

    nc.to_json_bytes = patched


def host_split_planes(proto: np.ndarray) -> np.ndarray:
    """Split the f32 table into hi/mid/lo bf16 planes (sum reconstructs every
    f32 exactly) laid out at partitions 0/32/64 with zero pads."""
    import ml_dtypes

    bf = ml_dtypes.bfloat16
    x = proto.astype(np.float32).reshape(N_PROTO, D)
    hi = x.astype(bf)
    r1 = x - hi.astype(np.float32)
    mid = r1.astype(bf)
    r2 = r1 - mid.astype(np.float32)
    lo = r2.astype(bf)
    planes = np.zeros((KDIM, D), dtype=bf)
    planes[0:N_PROTO] = hi
    planes[GP : GP + N_PROTO] = mid
    planes[2 * GP : 2 * GP + N_PROTO] = lo
    return planes


def host_onehot_planes(lbl_rows: np.ndarray) -> np.ndarray:
    """One-hot of the 1344 per-core labels, stacked three times along the
    contraction dim at partitions 0/32/64 (matching host_split_planes), as
    bf16. oh[g*GP + k, i] = 1 if lbl[i] == k else 0; pad rows are zero."""
    import ml_dtypes

    oh = np.zeros((KDIM, ROWS), dtype=ml_dtypes.bfloat16)
    hot = (np.arange(N_PROTO)[:, None] == lbl_rows[None, :]).astype(
        ml_dtypes.bfloat16
    )
    for g in range(3):
        oh[g * GP : g * GP + N_PROTO] = hot
    return oh


def host_split_planes_dense(proto: np.ndarray) -> np.ndarray:
    """hi/mid/lo bf16 planes packed densely at partitions 0/25/50 (K=75).
    Sum still reconstructs every f32 exactly; host-built stationary/moving
    operands don't need the 32-aligned group starts the on-device splitter
    required."""
    import ml_dtypes

    bf = ml_dtypes.bfloat16
    x = proto.astype(np.float32).reshape(N_PROTO, D)
    hi = x.astype(bf)
    r1 = x - hi.astype(np.float32)
    mid = r1.astype(bf)
    r2 = r1 - mid.astype(np.float32)
    lo = r2.astype(bf)
    planes = np.zeros((KDENSE, D), dtype=bf)
    planes[0:N_PROTO] = hi
    planes[N_PROTO : 2 * N_PROTO] = mid
    planes[2 * N_PROTO : 3 * N_PROTO] = lo
    return planes


def host_onehot_dense(lbl_rows: np.ndarray) -> np.ndarray:
    """One-hot stacked three times densely (partitions 0/25/50, K=75)."""
    import ml_dtypes

    oh = np.zeros((KDENSE, ROWS), dtype=ml_dtypes.bfloat16)
    hot = (np.arange(N_PROTO)[:, None] == lbl_rows[None, :]).astype(
        ml_dtypes.bfloat16
    )
    for g in range(3):
        oh[g * N_PROTO : (g + 1) * N_PROTO] = hot
    return oh


def host_onehot_v4(lbl_rows: np.ndarray) -> np.ndarray:
    """One-hot for the v4 permuted layout: level j's 128 columns map
    partition p -> output row row(p, j); non-participating (p, j) columns
    stay zero. Stacked at partitions 0/32/64 like host_onehot_planes."""
    import ml_dtypes

    oh = np.zeros((KDIM, MAX_S * 128), dtype=ml_dtypes.bfloat16)
    bases = _run_bases()
    for (p0, np_, s), base in zip(RUNS, bases):
        for j in range(s):
            rows = base + np.arange(np_) * s + j
            cols = j * 128 + p0 + np.arange(np_)
            lb = lbl_rows[rows]
            for g in range(3):
                oh[g * GP + lb, cols] = 1
    return oh


def v4_perm() -> np.ndarray:
    """perm[k] = output row held at (partition-major position k) — i.e. the
    inverse mapping used to validate layout; row(p, j) enumeration."""
    bases = _run_bases()
    perm = np.empty(ROWS, dtype=np.int64)
    i = 0
    for (p0, np_, s), base in zip(RUNS, bases):
        for pi in range(np_):
            for j in range(s):
                perm[i] = base + pi * s + j
                i += 1
    return perm


def build_nc_v4() -> bass.Bass:
    """v3 with the RUNS row layout and streaming per-(run,level) output
    DMAs. Input DMAs are chunked so the first matmul starts as soon as the
    first planes columns + level-0 one-hot land."""
    f32 = mybir.dt.float32
    bf16 = mybir.dt.bfloat16

    nc = bass.Bass()
    oh_in = nc.dram_tensor("oh", [KDIM, MAX_S * 128], bf16, kind="ExternalInput")
    planes_in = nc.dram_tensor("planes", [KDIM, D], bf16, kind="ExternalInput")
    out = nc.dram_tensor("out", [ROWS, D], f32, kind="ExternalOutput")

    n_col_tiles = (D + COL_TILE - 1) // COL_TILE
    n_pairs = (n_col_tiles + 1) // 2
    bases = _run_bases()

    with TileContext(nc) as tc:
        with (
            tc.tile_pool(name="const", bufs=1) as cpool,
            tc.tile_pool(name="psum", bufs=4, space="PSUM") as ppool,
        ):
            planes = cpool.tile([KDIM, D], bf16)
            oh = cpool.tile([KDIM, MAX_S * 128], bf16)
            ot = cpool.tile([ROW_TILE, MAX_S * D], f32)

            # first-needed slices first: planes cols for col-pair 0 and the
            # level-0 one-hot; the rest queues behind them
            nc.sync.dma_start(
                out=planes[:, : 2 * COL_TILE], in_=planes_in[:, : 2 * COL_TILE]
            )
            nc.sync.dma_start(out=oh[:, :128], in_=oh_in[:, :128])
            nc.sync.dma_start(
                out=planes[:, 2 * COL_TILE :], in_=planes_in[:, 2 * COL_TILE :]
            )
            nc.sync.dma_start(out=oh[:, 128:], in_=oh_in[:, 128:])

            # per-run flat DRAM views [np, s*D]: partition-stride s*D rows,
            # slots contiguous per partition (consecutive output rows)
            out_v = {}
            for (p0, np_, s), base in zip(RUNS, bases):
                out_v[p0] = out[base : base + np_ * s, :].rearrange(
                    "(p s) d -> p (s d)", s=s
                )

            for j in range(MAX_S):
                oh_sl = oh[:, j * 128 : (j + 1) * 128]
                for cp in range(n_pairs):
                    c0 = 2 * cp * COL_TILE
                    cw = min(2 * COL_TILE, D - c0)
                    ps = ppool.tile([ROW_TILE, 2 * COL_TILE], f32)
                    for h in range(2):
                        hw = min(COL_TILE, cw - h * COL_TILE)
                        if hw <= 0:
                            break
                        nc.tensor.matmul(
                            ps[:, h * COL_TILE : h * COL_TILE + hw],
                            oh_sl,
                            planes[:, c0 + h * COL_TILE : c0 + h * COL_TILE + hw],
                            start=True,
                            stop=True,
                        )
                    dst = ot[:, j * D + c0 : j * D + c0 + cw]
                    if cp % 2 == 1:
                        nc.scalar.copy(out=dst, in_=ps[:, :cw])
                    else:
                        nc.vector.tensor_copy(out=dst, in_=ps[:, :cw])
                    if j == 0:
                        # level 0 streams per col-pair for the big run so
                        # writes start immediately
                        p0, np_, s = RUNS[1]
                        nc.sync.dma_start(
                            out=out_v[p0][:, c0 : c0 + cw],
                            in_=ot[p0 : p0 + np_, c0 : c0 + cw],
                        )
                if j == 0:
                    for ri, (p0, np_, s) in enumerate(RUNS):
                        if ri == 1:
                            continue
                        nc.sync.dma_start(
                            out=out_v[p0][:, :D],
                            in_=ot[p0 : p0 + np_, :D],
                        )
                else:
                    for p0, np_, s in RUNS:
                        if j >= s:
                            continue
                        if p0 == RUNS[1][0]:
                            # big run: one DMA per level
                            nc.sync.dma_start(
                                out=out_v[p0][:, j * D : (j + 1) * D],
                                in_=ot[p0 : p0 + np_, j * D : (j + 1) * D],
                            )
                        elif j % 2 == 0:
                            # pair [j-1, j], emitted once both are computed
                            nc.sync.dma_start(
                                out=out_v[p0][:, (j - 1) * D : (j + 1) * D],
                                in_=ot[p0 : p0 + np_, (j - 1) * D : (j + 1) * D],
                            )
                        elif j == s - 1:
                            # odd final level (even slot count)
                            nc.sync.dma_start(
                                out=out_v[p0][:, j * D : (j + 1) * D],
                                in_=ot[p0 : p0 + np_, j * D : (j + 1) * D],
                            )
    _install_multiwait_splitter(nc)
    return nc


def build_nc_v3() -> bass.Bass:
    """Everything label-dependent precomputed on host: inputs are the bf16
    one-hot planes [96, 1344] and the bf16 table planes [96, D]. On device:
    chunked input DMAs split across both HWDGE rings (SP + ACT) so the
    first matmul's operands land ASAP, one 128x512 matmul per output
    subtile, PSUM->SBUF copies alternating DVE/ACT, per-col-pair output
    DMAs for the first row tile so the HBM write stream starts early.
    All output DMAs keep 128 descriptors (one per partition) so the
    positional round-robin over the 16 SDMA engines stays balanced."""
    f32 = mybir.dt.float32
    bf16 = mybir.dt.bfloat16

    nc = bass.Bass()
    oh_in = nc.dram_tensor("oh", [KD, ROWS], bf16, kind="ExternalInput")
    planes_in = nc.dram_tensor("planes", [KD, D], bf16, kind="ExternalInput")
    out = nc.dram_tensor("out", [ROWS, D], f32, kind="ExternalOutput")

    RT = ROW_TILE
    n_row_tiles = (ROWS + RT - 1) // RT
    n_col_tiles = (D + COL_TILE - 1) // COL_TILE
    n_pairs = (n_col_tiles + 1) // 2

    with TileContext(nc) as tc:
        with (
            tc.tile_pool(name="const", bufs=1) as cpool,
            tc.tile_pool(name="psum", bufs=4, space="PSUM") as ppool,
            tc.tile_pool(name="outp", bufs=n_row_tiles) as opool,
        ):
            oh = cpool.tile([KD, ROWS], bf16)
            planes = cpool.tile([KD, D], bf16)
            # PE starts HAM-throttled at half clock and releases only after
            # ~4us of sustained busy. Warm it on scratch data while the
            # input DMAs are in flight so the real matmuls run full-rate.
            warm_st = cpool.tile([KD, ROW_TILE], bf16)
            warm_mv = cpool.tile([KD, COL_TILE], bf16)
            # init on the otherwise-idle GpSimd engine so the PE warmup
            # doesn't wait on DVE/ACT
            nc.gpsimd.memset(warm_st, 0.0)
            nc.gpsimd.memset(warm_mv, 0.0)
            # first-needed slices first, split across the two HWDGE rings
            # Input descriptor sizing matters: >=4KB per-partition chunks
            # run ~2x faster than 1KB ones, and the SP ring must clear
            # before the first output DMA or its packets queue behind the
            # inputs. Two medium planes chunks on SP (first covers the
            # first two col-pairs), the one-hot as a single DMA on ACT.
            nc.sync.dma_start(
                out=planes[:, : 2 * COL_TILE], in_=planes_in[:, : 2 * COL_TILE]
            )
            nc.sync.dma_start(out=oh, in_=oh_in[:])
            nc.sync.dma_start(
                out=planes[:, 2 * COL_TILE :], in_=planes_in[:, 2 * COL_TILE :]
            )
            wps = ppool.tile([ROW_TILE, 2 * COL_TILE], f32, tag="ps")
            for _ in range(4):
                nc.tensor.matmul(
                    wps[:, :COL_TILE], warm_st, warm_mv, start=True, stop=True
                )

            for r in range(n_row_tiles):
                pr = min(RT, ROWS - r * RT)
                ot = opool.tile([ROW_TILE, D], f32)
                oh_sl = oh[:, r * RT : r * RT + pr]
                for cp in range(n_pairs):
                    c0 = 2 * cp * COL_TILE
                    cw = min(2 * COL_TILE, D - c0)
                    ps = ppool.tile([ROW_TILE, 2 * COL_TILE], f32, tag="ps")
                    for h in range(2):
                        hw = min(COL_TILE, cw - h * COL_TILE)
                        if hw <= 0:
                            break
                        nc.tensor.matmul(
                            ps[:pr, h * COL_TILE : h * COL_TILE + hw],
                            oh_sl,
                            planes[:, c0 + h * COL_TILE : c0 + h * COL_TILE + hw],
                            start=True,
                            stop=True,
                        )
                        if r == 0:
                            # fine-grained first tile: copy per 512-col
                            # matmul, alternating engines, to cut the
                            # latency to the first output bytes
                            dsth = ot[:pr, c0 + h * COL_TILE : c0 + h * COL_TILE + hw]
                            srch = ps[:pr, h * COL_TILE : h * COL_TILE + hw]
                            if (2 * cp + h) % 2 == 1:
                                nc.scalar.copy(out=dsth, in_=srch)
                            else:
                                nc.vector.tensor_copy(out=dsth, in_=srch)
                    if r > 0:
                        dst = ot[:pr, c0 : c0 + cw]
                        if cp % 2 == 1:
                            nc.scalar.copy(out=dst, in_=ps[:pr, :cw])
                        else:
                            nc.vector.tensor_copy(out=dst, in_=ps[:pr, :cw])
                    if r == 0 and cp == 0:
                        for h2 in range(2):
                            nc.sync.dma_start(
                                out=out[
                                    0:pr,
                                    c0 + h2 * COL_TILE : c0 + (h2 + 1) * COL_TILE,
                                ],
                                in_=ot[
                                    :pr,
                                    c0 + h2 * COL_TILE : c0 + (h2 + 1) * COL_TILE,
                                ],
                            )
                    elif r == 0:
                        # stream the first computed row tile per col pair so
                        # the write pipe stays busy as soon as bytes exist
                        nc.sync.dma_start(
                            out=out[0:pr, c0 : c0 + cw],
                            in_=ot[:pr, c0 : c0 + cw],
                        )
                    elif r == 1 and cp % 2 == 1:
                        # half-tile DMAs for the second row keep the early
                        # queue deep while the pipeline ramps
                        b0 = 2 * (cp - 1) * COL_TILE
                        nc.sync.dma_start(
                            out=out[RT : RT + pr, b0 : c0 + cw],
                            in_=ot[:pr, b0 : c0 + cw],
                        )
                if r > 1:
                    nc.sync.dma_start(
                        out=out[r * RT : r * RT + pr, :], in_=ot[:pr, :]
                    )
    _install_multiwait_splitter(nc)
    return nc


def build_nc_v2() -> bass.Bass:
    """Gather as one-hot @ planes matmul, K=96 (three bf16 planes of the
    table stacked along the contraction dim, pre-split on host). One matmul
    per 128x512 output tile; PSUM->SBUF copies alternate DVE/ACT; one DMA
    per 128-row tile."""
    f32 = mybir.dt.float32
    bf16 = mybir.dt.bfloat16
    i32 = mybir.dt.int32

    nc = bass.Bass()
    lbl = nc.dram_tensor("lbl", [1, ROWS], bf16, kind="ExternalInput")
    planes_in = nc.dram_tensor("planes", [KDIM, D], bf16, kind="ExternalInput")
    out = nc.dram_tensor("out", [ROWS, D], f32, kind="ExternalOutput")

    n_row_tiles = (ROWS + ROW_TILE - 1) // ROW_TILE
    n_col_tiles = (D + COL_TILE - 1) // COL_TILE
    OH_CHUNK = 448
    n_oh_chunks = (ROWS + OH_CHUNK - 1) // OH_CHUNK

    with TileContext(nc) as tc:
        with (
            tc.tile_pool(name="const", bufs=1) as cpool,
            tc.tile_pool(name="psum", bufs=4, space="PSUM") as ppool,
            tc.tile_pool(name="outp", bufs=11) as opool,
        ):
            lblsb = cpool.tile([1, ROWS], bf16)
            # tiny label vector rides the ACT ring so it lands immediately
            # instead of queueing behind the planes packets on SP
            nc.scalar.dma_start(out=lblsb, in_=lbl[:])

            planes = cpool.tile([KDIM, D], bf16)
            # two fat chunks (>=2KB per-partition descriptors) instead of
            # eight 1KB-descriptor column chunks: ~2x input drain rate, and
            # the SP ring clears sooner for the first output DMA
            nc.sync.dma_start(
                out=planes[:, : 2 * COL_TILE], in_=planes_in[:, : 2 * COL_TILE]
            )
            nc.sync.dma_start(
                out=planes[:, 2 * COL_TILE :], in_=planes_in[:, 2 * COL_TILE :]
            )
            ones = cpool.tile([1, KDIM], bf16)
            nc.vector.memset(ones, 1.0)

            iota_i = cpool.tile([KDIM, 1], i32)
            nc.gpsimd.iota(iota_i, pattern=[[0, 1]], base=0, channel_multiplier=1)
            iota_q = cpool.tile([KDIM, 1], i32)
            nc.vector.tensor_scalar(
                out=iota_q, in0=iota_i, scalar1=GP - 1, scalar2=None,
                op0=mybir.AluOpType.bitwise_and,
            )
            iota_m = cpool.tile([KDIM, 1], i32)
            nc.vector.tensor_scalar(
                out=iota_m, in0=iota_q, scalar1=N_PROTO, scalar2=None,
                op0=mybir.AluOpType.min,
            )
            iota_f = cpool.tile([KDIM, 1], f32)
            nc.vector.tensor_copy(out=iota_f, in_=iota_m)

            # broadcast labels to 96 partitions on the (idle) PE: ones^T @ lbl,
            # then compare against the per-partition group-local iota
            oh = cpool.tile([KDIM, ROWS], bf16)
            for ch in range(n_oh_chunks):
                cw = min(OH_CHUNK, ROWS - ch * OH_CHUNK)
                pb = ppool.tile([ROW_TILE, COL_TILE], f32, tag="ps")
                nc.tensor.matmul(
                    pb[:KDIM, :cw],
                    ones[0:1, :],
                    lblsb[0:1, ch * OH_CHUNK : ch * OH_CHUNK + cw],
                    start=True,
                    stop=True,
                )
                nc.vector.tensor_scalar(
                    out=oh[:, ch * OH_CHUNK : ch * OH_CHUNK + cw],
                    in0=pb[:KDIM, :cw],
                    scalar1=iota_f[:, 0:1],
                    scalar2=None,
                    op0=mybir.AluOpType.is_equal,
                )

            n_pairs = (n_col_tiles + 1) // 2
            for r in range(n_row_tiles):
                pr = min(ROW_TILE, ROWS - r * ROW_TILE)
                ot = opool.tile([ROW_TILE, D], f32)
                oh_sl = oh[:, r * ROW_TILE : r * ROW_TILE + pr]
                for cp in range(n_pairs):
                    c0 = 2 * cp * COL_TILE
                    cw = min(2 * COL_TILE, D - c0)
                    ps = ppool.tile([ROW_TILE, 2 * COL_TILE], f32)
                    for h in range(2):
                        hw = min(COL_TILE, cw - h * COL_TILE)
                        if hw <= 0:
                            break
                        nc.tensor.matmul(
                            ps[:pr, h * COL_TILE : h * COL_TILE + hw],
                            oh_sl,
                            planes[:, c0 + h * COL_TILE : c0 + h * COL_TILE + hw],
                            start=True,
                            stop=True,
                        )
                    dst = ot[:pr, c0 : c0 + cw]
                    if cp % 2 == 1:
                        nc.scalar.copy(out=dst, in_=ps[:pr, :cw])
                    else:
                        nc.vector.tensor_copy(out=dst, in_=ps[:pr, :cw])
                    if r == 0 and cp in (0, 1):
                        # prime the output-DMA stream before the tile finishes
                        nc.sync.dma_start(
                            out=out[0:pr, c0 : c0 + cw],
                            in_=ot[:pr, c0 : c0 + cw],
                        )
                if r == 0:
                    nc.sync.dma_start(
                        out=out[0:pr, 4 * COL_TILE :],
                        in_=ot[:pr, 4 * COL_TILE :],
                    )
                else:
                    nc.sync.dma_start(
                        out=out[r * ROW_TILE : r * ROW_TILE + pr, :], in_=ot[:pr, :]
                    )
    _install_multiwait_splitter(nc)
    return nc


def build_nc_k75() -> bass.Bass:
    """One matmul per output tile: stationary is the 25-row one-hot stacked
    three times along the contraction dim, the moving operand is the
    hi/mid/lo bf16 table planes stacked the same way. PSUM accumulates
    hi+mid+lo in fp32 in a single pass -> bit-exact f32 gather.

    Compute-engine SBUF accesses must start at a 32-aligned partition, so the
    three 25-row groups sit at partitions 0/32/64 (K=96). Pad partitions:
    one-hot rows compare labels against 25 (never matches -> 0), plane pad
    rows are zeroed via DMA so 0*0 keeps PSUM clean."""
    f32 = mybir.dt.float32
    bf16 = mybir.dt.bfloat16
    i32 = mybir.dt.int32
    GP = 32                  # partition stride between plane groups
    P3 = 3 * GP              # 96 = contraction dim incl. pads

    nc = bass.Bass()
    lbl = nc.dram_tensor("lbl", [1, ROWS], f32, kind="ExternalInput")
    proto = nc.dram_tensor("proto", [N_PROTO, D], f32, kind="ExternalInput")
    out = nc.dram_tensor("out", [ROWS, D], f32, kind="ExternalOutput")

    n_row_tiles = (ROWS + ROW_TILE - 1) // ROW_TILE
    n_col_tiles = (D + COL_TILE - 1) // COL_TILE

    with TileContext(nc) as tc:
        with (
            tc.tile_pool(name="const", bufs=1) as cpool,
            tc.tile_pool(name="psum", bufs=8, space="PSUM") as ppool,
            tc.tile_pool(name="outp", bufs=4) as opool,
        ):
            tbl75 = cpool.tile([P3, D], f32)
            lbl75 = cpool.tile([P3, ROWS], f32)
            for g in range(3):
                sl = slice(g * GP, g * GP + N_PROTO)
                nc.sync.dma_start(out=tbl75[sl, :], in_=proto[:])
                nc.sync.dma_start(
                    out=lbl75[g * GP : (g + 1) * GP, :],
                    in_=lbl[0].partition_broadcast(GP),
                )

            iota_i = cpool.tile([P3, 1], i32)
            nc.gpsimd.iota(iota_i, pattern=[[0, 1]], base=0, channel_multiplier=1)
            # group-local index, pads clamp to 25 which no label ever equals
            iota_q = cpool.tile([P3, 1], i32)
            nc.vector.tensor_scalar(
                out=iota_q, in0=iota_i, scalar1=GP - 1, scalar2=None,
                op0=mybir.AluOpType.bitwise_and,
            )
            iota_m = cpool.tile([P3, 1], i32)
            nc.vector.tensor_scalar(
                out=iota_m, in0=iota_q, scalar1=N_PROTO, scalar2=None,
                op0=mybir.AluOpType.min,
            )
            iota_f = cpool.tile([P3, 1], f32)
            nc.vector.tensor_copy(out=iota_f, in_=iota_m)

            oh = cpool.tile([P3, ROWS], bf16)
            nc.vector.tensor_scalar(
                out=oh, in0=lbl75, scalar1=iota_f[:, 0:1], scalar2=None,
                op0=mybir.AluOpType.is_equal,
            )

            # planes: partitions 0-24 hi, 32-56 mid, 64-88 lo (bf16, RN)
            planes = cpool.tile([P3, D], bf16)
            scrA = cpool.tile([P3, D], f32)
            scrB = cpool.tile([P3, D], f32)
            zpad = cpool.tile([GP - N_PROTO, D], bf16)
            nc.vector.memset(zpad, 0.0)
            for g in range(3):
                nc.sync.dma_start(
                    out=planes[g * GP + N_PROTO : (g + 1) * GP, :], in_=zpad
                )
            s0 = slice(0, N_PROTO)
            s1 = slice(GP, GP + N_PROTO)
            s2 = slice(2 * GP, 2 * GP + N_PROTO)
            # hi plane
            nc.vector.tensor_copy(out=planes[s0, :], in_=tbl75[s0, :])
            # mid plane: cast(x - f32(bf16(x)))
            nc.vector.tensor_copy(out=planes[s1, :], in_=tbl75[s1, :])
            nc.vector.tensor_copy(out=scrA[s1, :], in_=planes[s1, :])
            nc.vector.tensor_sub(out=planes[s1, :], in0=tbl75[s1, :], in1=scrA[s1, :])
            # lo plane: r1 = x - hi_f; mid = bf16(r1); lo = bf16(r1 - f32(mid))
            nc.vector.tensor_copy(out=planes[s2, :], in_=tbl75[s2, :])
            nc.vector.tensor_copy(out=scrA[s2, :], in_=planes[s2, :])
            nc.vector.tensor_sub(out=scrB[s2, :], in0=tbl75[s2, :], in1=scrA[s2, :])
            nc.vector.tensor_copy(out=planes[s2, :], in_=scrB[s2, :])
            nc.vector.tensor_copy(out=scrA[s2, :], in_=planes[s2, :])
            nc.vector.tensor_sub(out=planes[s2, :], in0=scrB[s2, :], in1=scrA[s2, :])

            for r in range(n_row_tiles):
                pr = min(ROW_TILE, ROWS - r * ROW_TILE)
                ot = opool.tile([ROW_TILE, D], f32)
                oh_sl = oh[:, r * ROW_TILE : r * ROW_TILE + pr]
                for c in range(n_col_tiles):
                    cn = min(COL_TILE, D - c * COL_TILE)
                    ps = ppool.tile([ROW_TILE, COL_TILE], f32)
                    nc.tensor.matmul(
                        ps[:pr, :cn],
                        oh_sl,
                        planes[:, c * COL_TILE : c * COL_TILE + cn],
                        start=True,
                        stop=True,
                    )
                    dst = ot[:pr, c * COL_TILE : c * COL_TILE + cn]
                    if c in (3, 7):
                        nc.scalar.copy(out=dst, in_=ps[:pr, :cn])
                    else:
                        nc.vector.tensor_copy(out=dst, in_=ps[:pr, :cn])
                nc.sync.dma_start(
                    out=out[r * ROW_TILE : r * ROW_TILE + pr, :], in_=ot[:pr, :]
                )
    _install_multiwait_splitter(nc)
    return nc


def build_nc(mode: str = _MODE) -> bass.Bass:
    if mode == "v4":
        return build_nc_v4()
    if mode == "v3":
        return build_nc_v3()
    if mode == "v2":
        return build_nc_v2()
    if mode == "k75":
        return build_nc_k75()
    f32 = mybir.dt.float32
    bf16 = mybir.dt.bfloat16

    nc = bass.Bass()
    lbl = nc.dram_tensor("lbl", [1, ROWS], f32, kind="ExternalInput")
    proto = nc.dram_tensor("proto", [N_PROTO, D], f32, kind="ExternalInput")
    out = nc.dram_tensor("out", [ROWS, D], f32, kind="ExternalOutput")

    n_row_tiles = (ROWS + ROW_TILE - 1) // ROW_TILE
    n_col_tiles = (D + COL_TILE - 1) // COL_TILE

    with TileContext(nc) as tc:
        with (
            tc.tile_pool(name="const", bufs=1) as cpool,
            tc.tile_pool(name="psum", bufs=8, space="PSUM") as ppool,
            tc.tile_pool(name="outp", bufs=4) as opool,
        ):
            tbl = cpool.tile([N_PROTO, D], f32)
            nc.sync.dma_start(out=tbl, in_=proto[:])

            lblb = cpool.tile([N_PROTO, ROWS], f32)
            nc.sync.dma_start(out=lblb, in_=lbl[0].partition_broadcast(N_PROTO))

            iot = cpool.tile([N_PROTO, 1], f32)
            nc.gpsimd.iota(
                iot,
                pattern=[[0, 1]],
                base=0,
                channel_multiplier=1,
                allow_small_or_imprecise_dtypes=True,
            )

            oh_dt = f32 if mode in ("f32", "f32r") else bf16
            oh = cpool.tile([N_PROTO, ROWS], oh_dt)
            nc.vector.tensor_scalar(
                out=oh,
                in0=lblb,
                scalar1=iot[:, 0:1],
                scalar2=None,
                op0=mybir.AluOpType.is_equal,
            )

            if mode in ("f32", "f32r"):
                planes = [tbl]
            else:
                # Exact f32 = hi + mid + lo, each bf16 (RN cast at each step).
                hi = cpool.tile([N_PROTO, D], bf16)
                nc.vector.tensor_copy(out=hi, in_=tbl)
                hi_f = cpool.tile([N_PROTO, D], f32)
                nc.vector.tensor_copy(out=hi_f, in_=hi)
                r1 = cpool.tile([N_PROTO, D], f32)
                nc.vector.tensor_sub(out=r1, in0=tbl, in1=hi_f)
                mid = cpool.tile([N_PROTO, D], bf16)
                nc.vector.tensor_copy(out=mid, in_=r1)
                planes = [hi, mid]
                if mode == "bf16x3":
                    mid_f = cpool.tile([N_PROTO, D], f32)
                    nc.vector.tensor_copy(out=mid_f, in_=mid)
                    r2 = cpool.tile([N_PROTO, D], f32)
                    nc.vector.tensor_sub(out=r2, in0=r1, in1=mid_f)
                    lo = cpool.tile([N_PROTO, D], bf16)
                    nc.vector.tensor_copy(out=lo, in_=r2)
                    planes.append(lo)

            for r in range(n_row_tiles):
                pr = min(ROW_TILE, ROWS - r * ROW_TILE)
                ot = opool.tile([ROW_TILE, D], f32)
                oh_sl = oh[:, r * ROW_TILE : r * ROW_TILE + pr]
                if mode == "f32r":
                    oh_sl = oh_sl.bitcast(mybir.dt.float32r)
                for c in range(n_col_tiles):
                    cn = min(COL_TILE, D - c * COL_TILE)
                    ps = ppool.tile([ROW_TILE, COL_TILE], f32)
                    for pi, plane in enumerate(planes):
                        rhs = plane[:, c * COL_TILE : c * COL_TILE + cn]
                        if mode == "f32r":
                            rhs = rhs.bitcast(mybir.dt.float32r)
                        nc.tensor.matmul(
                            ps[:pr, :cn],
                            oh_sl,
                            rhs,
                            start=(pi == 0),
                            stop=(pi == len(planes) - 1),
                        )
                    nc.vector.tensor_copy(
                        out=ot[:pr, c * COL_TILE : c * COL_TILE + cn],
                        in_=ps[:pr, :cn],
                    )
                nc.sync.dma_start(
                    out=out[r * ROW_TILE : r * ROW_TILE + pr, :], in_=ot[:pr, :]
                )
    _install_multiwait_splitter(nc)
    return nc


_NC_CACHE: dict[str, bass.Bass] = {}


def _get_nc(mode: str) -> bass.Bass:
    if mode not in _NC_CACHE:
        _NC_CACHE[mode] = build_nc(mode)
    return _NC_CACHE[mode]


def run(inputs, labels, prototypes, mode: str = _MODE, **spmd_kwargs):
    """Run the kernel; returns (output, BassKernelResults)."""
    lbl = np.asarray(labels).reshape(B, L)
    proto = np.ascontiguousarray(
        np.asarray(prototypes, dtype=np.float32).reshape(N_PROTO, D)
    )
    in_maps = []
    if mode in ("v3", "v4"):
        if mode == "v3":
            if V3_DENSE:
                planes = host_split_planes_dense(proto)
                mk = host_onehot_dense
            else:
                planes = host_split_planes(proto)
                mk = host_onehot_planes
        else:
            planes = host_split_planes(proto)
            mk = host_onehot_v4
        for c in range(N_CORES):
            lr = lbl[c * B_PER_CORE : (c + 1) * B_PER_CORE].reshape(ROWS)
            in_maps.append({"oh": mk(lr), "planes": planes})
    else:
        if mode == "v2":
            import ml_dtypes

            table_input = {"planes": host_split_planes(proto)}
            lbl_dt = ml_dtypes.bfloat16
        else:
            table_input = {"proto": proto}
            lbl_dt = np.float32
        for c in range(N_CORES):
            lf = (
                lbl[c * B_PER_CORE : (c + 1) * B_PER_CORE]
                .reshape(1, ROWS)
                .astype(lbl_dt)
            )
            in_maps.append({"lbl": lf, **table_input})
    res = run_bass_kernel_spmd(
        _get_nc(mode), in_maps, core_ids=list(range(N_CORES)), **spmd_kwargs
    )
    outs = [
        r["out"].reshape(B_PER_CORE, L, NCHAN, T, F) for r in res.results
    ]
    return np.concatenate(outs, axis=0), res


def kernel(inputs, labels, prototypes):
    out, _ = run(inputs, labels, prototypes)
    return out



# revision 38
# speedup vs baseline: 1.0717x; 1.0717x over previous
"""Embedding-lookup kernel for Trainium2 (Bass/Tile), 8-core data-parallel.

Problem: out[b, l] = prototypes[labels[b, l]]
  inputs     (512, 21, 1, 29, 129) f32  -- unused except for batch size
  labels     (512, 21) int64            -- values in [0, 25)
  prototypes (25, 1, 29, 129) f32
  out        (512, 21, 1, 29, 129) f32  (~161 MB)

Strategy (memory regime): shard the batch dim across 8 cores (1344 lookups
per core, 20.1 MB of output writes each). Per core the gather runs as
one-hot @ table matmuls on the PE, streaming PSUM -> SBUF -> DRAM so HBM
traffic is write-only. Default mode "v2": the f32 table is host-split into
three bf16 planes (hi/mid/lo at partitions 0/32/64, K=96) whose sum
reconstructs every f32 exactly; the one-hot is built on device from the
label vector (PE ones-broadcast + iota/is_equal) -- with 0/1 weights the
gather is bit-exact. Inputs load as two fat planes chunks on the SP HWDGE
ring plus the tiny label DMA on the ACT ring.

Schedule notes (from perfetto/NTFF traces on the axon trn2 pool):
 - the 16 SDMA engines cap at ~24.7 GB/s each (~420 GB/s/core steady);
   output DMAs keep 128 descriptors (1 per partition) because descriptors
   round-robin positionally over engines and odd counts skew the load
 - input DMAs use >=2KB per-partition descriptors and ride both HWDGE
   rings (planes on SP, one-hot on ACT); the first output DMA otherwise
   queues behind input packets in the ring FIFO
 - the PE comes up HAM-clock-gated at half rate; dummy matmuls on scratch
   SBUF while inputs are in flight pull the full-rate transition earlier
 - the first row tile streams per 512-col matmul (copy + DMA per half
   pair) to start the write stream ~4 us earlier
Measured (mode v2): 67.8-68.3 us HW exec in quiet windows, ~74-80 us when
the shared chip/HBM or DMA engine 15 is contended; bit-exact vs the f32
reference. Restructured variants (v3 host-built one-hot, v4 permuted
layout) measured slower and stay selectable for reference.
"""

import json

import numpy as np

import concourse.bass as bass
import concourse.mybir as mybir
from concourse.tile import TileContext
from concourse.bass_utils import run_bass_kernel_spmd

B, L, NCHAN, T, F = 512, 21, 1, 29, 129
D = NCHAN * T * F            # 3741 features per prototype
N_PROTO = 25
N_CORES = 8
B_PER_CORE = B // N_CORES    # 64
ROWS = B_PER_CORE * L        # 1344 lookups per core

ROW_TILE = 128               # output rows per matmul (PSUM partition dim)
COL_TILE = 512               # output cols per matmul (one PSUM bank of f32)

# "v2" (exact; host-split bf16 planes, on-device one-hot build) is the
# default: across today's A/B sessions it holds a 67.9-68.3 us band and
# rarely triggers the slow-DMA-engine mode, beating every restructuring
# attempt ("v3" host-built one-hot + tuned schedule: 69.1+ us and ~50%
# slow-mode; "v4" permuted row layout: worse). "k75"/"bf16x3" are
# on-device splits, "f32"/"f32r" probes only.
_MODE = "v2"

# v4 layout: partition p holds RUNS[r] consecutive output rows as
# contiguous slots. SBUF port 15 serves partitions 92-95/124-127; its DMA
# engine is intermittently ~20% slower, so those partitions get 8 slots
# while the rest get 10-11. (start_partition, n_partitions, slots)
RUNS = [
    (0, 32, 10),
    (32, 60, 11),
    (92, 4, 8),
    (96, 20, 11),
    (116, 8, 10),
    (124, 4, 8),
]
MAX_S = 11
assert sum(np_ * s for _, np_, s in RUNS) == ROWS


def _run_bases() -> list[int]:
    bases, acc = [], 0
    for _, np_, s in RUNS:
        bases.append(acc)
        acc += np_ * s
    return bases




GP = 32                  # partition stride between the three plane groups
KDIM = 3 * GP            # 96 = matmul contraction dim incl. zero pads
KDENSE = 3 * N_PROTO     # 75 = dense contraction dim (host-packed planes)
import os as _os
V3_DENSE = _os.environ.get("V3_DENSE", "1") == "1"
KD = KDENSE if V3_DENSE else KDIM


def _split_multiwaits(bir: dict) -> int:
    """This walrus build allows at most one sync-wait per instruction on
    several instruction encodings; Tile attaches one wait per dependency.
    Hoist every wait of a multi-wait instruction into its own EventSemaphore
    (the encoding `wait_ge` uses) inserted directly before it on the same
    engine. Returns the number of instructions split."""
    n_split = 0
    ctr = 0
    for f in bir["functions"]:
        for blk in f["blocks"]:
            insts = blk["instructions"]
            out = []
            for inst in insts:
                si = inst.get("sync_info")
                waits = (si or {}).get("on_wait") or []
                if len(waits) > 1:
                    n_split += 1
                    for w in waits:
                        ctr += 1
                        out.append(
                            {
                                "debug": inst.get("debug", 0),
                                "engine": inst["engine"],
                                "ins": [],
                                "outs": [],
                                "name": f"mwsplit-{ctr}",
                                "opcode": "EventSemaphore",
                                "sync_info": {"on_update": [], "on_wait": [w]},
                            }
                        )
                    si["on_wait"] = []
                out.append(inst)
            blk["instructions"] = out
    return n_split


def _install_multiwait_splitter(nc: bass.Bass) -> None:
    orig = nc.to_json_bytes

    def patched() -> bytes:
        bir = json.loads(orig())
        _split_multiwaits(bir)
        return json.dumps(bir).encode()

    nc.to_json_bytes = patched


def host_split_planes(proto: np.ndarray) -> np.ndarray:
    """Split the f32 table into hi/mid/lo bf16 planes (sum reconstructs every
    f32 exactly) laid out at partitions 0/32/64 with zero pads."""
    import ml_dtypes

    bf = ml_dtypes.bfloat16
    x = proto.astype(np.float32).reshape(N_PROTO, D)
    hi = x.astype(bf)
    r1 = x - hi.astype(np.float32)
    mid = r1.astype(bf)
    r2 = r1 - mid.astype(np.float32)
    lo = r2.astype(bf)
    planes = np.zeros((KDIM, D), dtype=bf)
    planes[0:N_PROTO] = hi
    planes[GP : GP + N_PROTO] = mid
    planes[2 * GP : 2 * GP + N_PROTO] = lo
    return planes


def host_onehot_planes(lbl_rows: np.ndarray) -> np.ndarray:
    """One-hot of the 1344 per-core labels, stacked three times along the
    contraction dim at partitions 0/32/64 (matching host_split_planes), as
    bf16. oh[g*GP + k, i] = 1 if lbl[i] == k else 0; pad rows are zero."""
    import ml_dtypes

    oh = np.zeros((KDIM, ROWS), dtype=ml_dtypes.bfloat16)
    hot = (np.arange(N_PROTO)[:, None] == lbl_rows[None, :]).astype(
        ml_dtypes.bfloat16
    )
    for g in range(3):
        oh[g * GP : g * GP + N_PROTO] = hot
    return oh


def host_split_planes_dense(proto: np.ndarray) -> np.ndarray:
    """hi/mid/lo bf16 planes packed densely at partitions 0/25/50 (K=75).
    Sum still reconstructs every f32 exactly; host-built stationary/moving
    operands don't need the 32-aligned group starts the on-device splitter
    required."""
    import ml_dtypes

    bf = ml_dtypes.bfloat16
    x = proto.astype(np.float32).reshape(N_PROTO, D)
    hi = x.astype(bf)
    r1 = x - hi.astype(np.float32)
    mid = r1.astype(bf)
    r2 = r1 - mid.astype(np.float32)
    lo = r2.astype(bf)
    planes = np.zeros((KDENSE, D), dtype=bf)
    planes[0:N_PROTO] = hi
    planes[N_PROTO : 2 * N_PROTO] = mid
    planes[2 * N_PROTO : 3 * N_PROTO] = lo
    return planes


def host_onehot_dense(lbl_rows: np.ndarray) -> np.ndarray:
    """One-hot stacked three times densely (partitions 0/25/50, K=75)."""
    import ml_dtypes

    oh = np.zeros((KDENSE, ROWS), dtype=ml_dtypes.bfloat16)
    hot = (np.arange(N_PROTO)[:, None] == lbl_rows[None, :]).astype(
        ml_dtypes.bfloat16
    )
    for g in range(3):
        oh[g * N_PROTO : (g + 1) * N_PROTO] = hot
    return oh


def host_onehot_v4(lbl_rows: np.ndarray) -> np.ndarray:
    """One-hot for the v4 permuted layout: level j's 128 columns map
    partition p -> output row row(p, j); non-participating (p, j) columns
    stay zero. Stacked at partitions 0/32/64 like host_onehot_planes."""
    import ml_dtypes

    oh = np.zeros((KDIM, MAX_S * 128), dtype=ml_dtypes.bfloat16)
    bases = _run_bases()
    for (p0, np_, s), base in zip(RUNS, bases):
        for j in range(s):
            rows = base + np.arange(np_) * s + j
            cols = j * 128 + p0 + np.arange(np_)
            lb = lbl_rows[rows]
            for g in range(3):
                oh[g * GP + lb, cols] = 1
    return oh


def v4_perm() -> np.ndarray:
    """perm[k] = output row held at (partition-major position k) — i.e. the
    inverse mapping used to validate layout; row(p, j) enumeration."""
    bases = _run_bases()
    perm = np.empty(ROWS, dtype=np.int64)
    i = 0
    for (p0, np_, s), base in zip(RUNS, bases):
        for pi in range(np_):
            for j in range(s):
                perm[i] = base + pi * s + j
                i += 1
    return perm


def build_nc_v4() -> bass.Bass:
    """v3 with the RUNS row layout and streaming per-(run,level) output
    DMAs. Input DMAs are chunked so the first matmul starts as soon as the
    first planes columns + level-0 one-hot land."""
    f32 = mybir.dt.float32
    bf16 = mybir.dt.bfloat16

    nc = bass.Bass()
    oh_in = nc.dram_tensor("oh", [KDIM, MAX_S * 128], bf16, kind="ExternalInput")
    planes_in = nc.dram_tensor("planes", [KDIM, D], bf16, kind="ExternalInput")
    out = nc.dram_tensor("out", [ROWS, D], f32, kind="ExternalOutput")

    n_col_tiles = (D + COL_TILE - 1) // COL_TILE
    n_pairs = (n_col_tiles + 1) // 2
    bases = _run_bases()

    with TileContext(nc) as tc:
        with (
            tc.tile_pool(name="const", bufs=1) as cpool,
            tc.tile_pool(name="psum", bufs=4, space="PSUM") as ppool,
        ):
            planes = cpool.tile([KDIM, D], bf16)
            oh = cpool.tile([KDIM, MAX_S * 128], bf16)
            ot = cpool.tile([ROW_TILE, MAX_S * D], f32)

            # first-needed slices first: planes cols for col-pair 0 and the
            # level-0 one-hot; the rest queues behind them
            nc.sync.dma_start(
                out=planes[:, : 2 * COL_TILE], in_=planes_in[:, : 2 * COL_TILE]
            )
            nc.sync.dma_start(out=oh[:, :128], in_=oh_in[:, :128])
            nc.sync.dma_start(
                out=planes[:, 2 * COL_TILE :], in_=planes_in[:, 2 * COL_TILE :]
            )
            nc.sync.dma_start(out=oh[:, 128:], in_=oh_in[:, 128:])

            # per-run flat DRAM views [np, s*D]: partition-stride s*D rows,
            # slots contiguous per partition (consecutive output rows)
            out_v = {}
            for (p0, np_, s), base in zip(RUNS, bases):
                out_v[p0] = out[base : base + np_ * s, :].rearrange(
                    "(p s) d -> p (s d)", s=s
                )

            for j in range(MAX_S):
                oh_sl = oh[:, j * 128 : (j + 1) * 128]
                for cp in range(n_pairs):
                    c0 = 2 * cp * COL_TILE
                    cw = min(2 * COL_TILE, D - c0)
                    ps = ppool.tile([ROW_TILE, 2 * COL_TILE], f32)
                    for h in range(2):
                        hw = min(COL_TILE, cw - h * COL_TILE)
                        if hw <= 0:
                            break
                        nc.tensor.matmul(
                            ps[:, h * COL_TILE : h * COL_TILE + hw],
                            oh_sl,
                            planes[:, c0 + h * COL_TILE : c0 + h * COL_TILE + hw],
                            start=True,
                            stop=True,
                        )
                    dst = ot[:, j * D + c0 : j * D + c0 + cw]
                    if cp % 2 == 1:
                        nc.scalar.copy(out=dst, in_=ps[:, :cw])
                    else:
                        nc.vector.tensor_copy(out=dst, in_=ps[:, :cw])
                    if j == 0:
                        # level 0 streams per col-pair for the big run so
                        # writes start immediately
                        p0, np_, s = RUNS[1]
                        nc.sync.dma_start(
                            out=out_v[p0][:, c0 : c0 + cw],
                            in_=ot[p0 : p0 + np_, c0 : c0 + cw],
                        )
                if j == 0:
                    for ri, (p0, np_, s) in enumerate(RUNS):
                        if ri == 1:
                            continue
                        nc.sync.dma_start(
                            out=out_v[p0][:, :D],
                            in_=ot[p0 : p0 + np_, :D],
                        )
                else:
                    for p0, np_, s in RUNS:
                        if j >= s:
                            continue
                        if p0 == RUNS[1][0]:
                            # big run: one DMA per level
                            nc.sync.dma_start(
                                out=out_v[p0][:, j * D : (j + 1) * D],
                                in_=ot[p0 : p0 + np_, j * D : (j + 1) * D],
                            )
                        elif j % 2 == 0:
                            # pair [j-1, j], emitted once both are computed
                            nc.sync.dma_start(
                                out=out_v[p0][:, (j - 1) * D : (j + 1) * D],
                                in_=ot[p0 : p0 + np_, (j - 1) * D : (j + 1) * D],
                            )
                        elif j == s - 1:
                            # odd final level (even slot count)
                            nc.sync.dma_start(
                                out=out_v[p0][:, j * D : (j + 1) * D],
                                in_=ot[p0 : p0 + np_, j * D : (j + 1) * D],
                            )
    _install_multiwait_splitter(nc)
    return nc


def build_nc_v3() -> bass.Bass:
    """Everything label-dependent precomputed on host: inputs are the bf16
    one-hot planes [96, 1344] and the bf16 table planes [96, D]. On device:
    chunked input DMAs split across both HWDGE rings (SP + ACT) so the
    first matmul's operands land ASAP, one 128x512 matmul per output
    subtile, PSUM->SBUF copies alternating DVE/ACT, per-col-pair output
    DMAs for the first row tile so the HBM write stream starts early.
    All output DMAs keep 128 descriptors (one per partition) so the
    positional round-robin over the 16 SDMA engines stays balanced."""
    f32 = mybir.dt.float32
    bf16 = mybir.dt.bfloat16

    nc = bass.Bass()
    oh_in = nc.dram_tensor("oh", [KD, ROWS], bf16, kind="ExternalInput")
    planes_in = nc.dram_tensor("planes", [KD, D], bf16, kind="ExternalInput")
    out = nc.dram_tensor("out", [ROWS, D], f32, kind="ExternalOutput")

    RT = ROW_TILE
    n_row_tiles = (ROWS + RT - 1) // RT
    n_col_tiles = (D + COL_TILE - 1) // COL_TILE
    n_pairs = (n_col_tiles + 1) // 2

    with TileContext(nc) as tc:
        with (
            tc.tile_pool(name="const", bufs=1) as cpool,
            tc.tile_pool(name="psum", bufs=4, space="PSUM") as ppool,
            tc.tile_pool(name="outp", bufs=n_row_tiles) as opool,
        ):
            oh = cpool.tile([KD, ROWS], bf16)
            planes = cpool.tile([KD, D], bf16)
            # PE starts HAM-throttled at half clock and releases only after
            # ~4us of sustained busy. Warm it on scratch data while the
            # input DMAs are in flight so the real matmuls run full-rate.
            warm_st = cpool.tile([KD, ROW_TILE], bf16)
            warm_mv = cpool.tile([KD, COL_TILE], bf16)
            # init on the otherwise-idle GpSimd engine so the PE warmup
            # doesn't wait on DVE/ACT
            nc.gpsimd.memset(warm_st, 0.0)
            nc.gpsimd.memset(warm_mv, 0.0)
            # first-needed slices first, split across the two HWDGE rings
            # Input descriptor sizing matters: >=4KB per-partition chunks
            # run ~2x faster than 1KB ones, and the SP ring must clear
            # before the first output DMA or its packets queue behind the
            # inputs. Two medium planes chunks on SP (first covers the
            # first two col-pairs), the one-hot as a single DMA on ACT.
            nc.sync.dma_start(
                out=planes[:, : 2 * COL_TILE], in_=planes_in[:, : 2 * COL_TILE]
            )
            nc.sync.dma_start(out=oh, in_=oh_in[:])
            nc.sync.dma_start(
                out=planes[:, 2 * COL_TILE :], in_=planes_in[:, 2 * COL_TILE :]
            )
            wps = ppool.tile([ROW_TILE, 2 * COL_TILE], f32, tag="ps")
            for _ in range(4):
                nc.tensor.matmul(
                    wps[:, :COL_TILE], warm_st, warm_mv, start=True, stop=True
                )

            for r in range(n_row_tiles):
                pr = min(RT, ROWS - r * RT)
                ot = opool.tile([ROW_TILE, D], f32)
                oh_sl = oh[:, r * RT : r * RT + pr]
                for cp in range(n_pairs):
                    c0 = 2 * cp * COL_TILE
                    cw = min(2 * COL_TILE, D - c0)
                    ps = ppool.tile([ROW_TILE, 2 * COL_TILE], f32, tag="ps")
                    for h in range(2):
                        hw = min(COL_TILE, cw - h * COL_TILE)
                        if hw <= 0:
                            break
                        nc.tensor.matmul(
                            ps[:pr, h * COL_TILE : h * COL_TILE + hw],
                            oh_sl,
                            planes[:, c0 + h * COL_TILE : c0 + h * COL_TILE + hw],
                            start=True,
                            stop=True,
                        )
                        if r == 0:
                            # fine-grained first tile: copy per 512-col
                            # matmul, alternating engines, to cut the
                            # latency to the first output bytes
                            dsth = ot[:pr, c0 + h * COL_TILE : c0 + h * COL_TILE + hw]
                            srch = ps[:pr, h * COL_TILE : h * COL_TILE + hw]
                            if (2 * cp + h) % 2 == 1:
                                nc.scalar.copy(out=dsth, in_=srch)
                            else:
                                nc.vector.tensor_copy(out=dsth, in_=srch)
                    if r > 0:
                        dst = ot[:pr, c0 : c0 + cw]
                        if cp % 2 == 1:
                            nc.scalar.copy(out=dst, in_=ps[:pr, :cw])
                        else:
                            nc.vector.tensor_copy(out=dst, in_=ps[:pr, :cw])
                    if r == 0 and cp == 0:
                        for h2 in range(2):
                            nc.sync.dma_start(
                                out=out[
                                    0:pr,
                                    c0 + h2 * COL_TILE : c0 + (h2 + 1) * COL_TILE,
                                ],
                                in_=ot[
                                    :pr,
                                    c0 + h2 * COL_TILE : c0 + (h2 + 1) * COL_TILE,
                                ],
                            )
                    elif r == 0:
                        # stream the first computed row tile per col pair so
                        # the write pipe stays busy as soon as bytes exist
                        nc.sync.dma_start(
                            out=out[0:pr, c0 : c0 + cw],
                            in_=ot[:pr, c0 : c0 + cw],
                        )
                    elif r == 1 and cp % 2 == 1:
                        # half-tile DMAs for the second row keep the early
                        # queue deep while the pipeline ramps
                        b0 = 2 * (cp - 1) * COL_TILE
                        nc.sync.dma_start(
                            out=out[RT : RT + pr, b0 : c0 + cw],
                            in_=ot[:pr, b0 : c0 + cw],
                        )
                if r > 1:
                    nc.sync.dma_start(
                        out=out[r * RT : r * RT + pr, :], in_=ot[:pr, :]
                    )
    _install_multiwait_splitter(nc)
    return nc


def build_nc_v2() -> bass.Bass:
    """Gather as one-hot @ planes matmul, K=96 (three bf16 planes of the
    table stacked along the contraction dim, pre-split on host). One matmul
    per 128x512 output tile; PSUM->SBUF copies alternate DVE/ACT; one DMA
    per 128-row tile."""
    f32 = mybir.dt.float32
    bf16 = mybir.dt.bfloat16
    i32 = mybir.dt.int32

    nc = bass.Bass()
    lbl = nc.dram_tensor("lbl", [1, ROWS], bf16, kind="ExternalInput")
    planes_in = nc.dram_tensor("planes", [KDIM, D], bf16, kind="ExternalInput")
    out = nc.dram_tensor("out", [ROWS, D], f32, kind="ExternalOutput")

    n_row_tiles = (ROWS + ROW_TILE - 1) // ROW_TILE
    n_col_tiles = (D + COL_TILE - 1) // COL_TILE
    OH_CHUNK = 448
    n_oh_chunks = (ROWS + OH_CHUNK - 1) // OH_CHUNK

    with TileContext(nc) as tc:
        with (
            tc.tile_pool(name="const", bufs=1) as cpool,
            tc.tile_pool(name="psum", bufs=4, space="PSUM") as ppool,
            tc.tile_pool(name="outp", bufs=11) as opool,
        ):
            lblsb = cpool.tile([1, ROWS], bf16)
            # tiny label vector rides the ACT ring so it lands immediately
            # instead of queueing behind the planes packets on SP
            nc.scalar.dma_start(out=lblsb, in_=lbl[:])

            planes = cpool.tile([KDIM, D], bf16)
            # two fat chunks (>=2KB per-partition descriptors) instead of
            # eight 1KB-descriptor column chunks: ~2x input drain rate, and
            # the SP ring clears sooner for the first output DMA
            nc.sync.dma_start(
                out=planes[:, : 2 * COL_TILE], in_=planes_in[:, : 2 * COL_TILE]
            )
            nc.sync.dma_start(
                out=planes[:, 2 * COL_TILE :], in_=planes_in[:, 2 * COL_TILE :]
            )
            ones = cpool.tile([1, KDIM], bf16)
            nc.vector.memset(ones, 1.0)

            iota_i = cpool.tile([KDIM, 1], i32)
            nc.gpsimd.iota(iota_i, pattern=[[0, 1]], base=0, channel_multiplier=1)
            iota_q = cpool.tile([KDIM, 1], i32)
            nc.vector.tensor_scalar(
                out=iota_q, in0=iota_i, scalar1=GP - 1, scalar2=None,
                op0=mybir.AluOpType.bitwise_and,
            )
            iota_m = cpool.tile([KDIM, 1], i32)
            nc.vector.tensor_scalar(
                out=iota_m, in0=iota_q, scalar1=N_PROTO, scalar2=None,
                op0=mybir.AluOpType.min,
            )
            iota_f = cpool.tile([KDIM, 1], f32)
            nc.vector.tensor_copy(out=iota_f, in_=iota_m)

            # broadcast labels to 96 partitions on the (idle) PE: ones^T @ lbl,
            # then compare against the per-partition group-local iota
            oh = cpool.tile([KDIM, ROWS], bf16)
            for ch in range(n_oh_chunks):
                cw = min(OH_CHUNK, ROWS - ch * OH_CHUNK)
                pb = ppool.tile([ROW_TILE, COL_TILE], f32, tag="ps")
                nc.tensor.matmul(
                    pb[:KDIM, :cw],
                    ones[0:1, :],
                    lblsb[0:1, ch * OH_CHUNK : ch * OH_CHUNK + cw],
                    start=True,
                    stop=True,
                )
                nc.vector.tensor_scalar(
                    out=oh[:, ch * OH_CHUNK : ch * OH_CHUNK + cw],
                    in0=pb[:KDIM, :cw],
                    scalar1=iota_f[:, 0:1],
                    scalar2=None,
                    op0=mybir.AluOpType.is_equal,
                )

            n_pairs = (n_col_tiles + 1) // 2
            for r in range(n_row_tiles):
                pr = min(ROW_TILE, ROWS - r * ROW_TILE)
                ot = opool.tile([ROW_TILE, D], f32)
                oh_sl = oh[:, r * ROW_TILE : r * ROW_TILE + pr]
                for cp in range(n_pairs):
                    c0 = 2 * cp * COL_TILE
                    cw = min(2 * COL_TILE, D - c0)
                    ps = ppool.tile([ROW_TILE, 2 * COL_TILE], f32)
                    for h in range(2):
                        hw = min(COL_TILE, cw - h * COL_TILE)
                        if hw <= 0:
                            break
                        nc.tensor.matmul(
                            ps[:pr, h * COL_TILE : h * COL_TILE + hw],
                            oh_sl,
                            planes[:, c0 + h * COL_TILE : c0 + h * COL_TILE + hw],
                            start=True,
                            stop=True,
                        )
                    dst = ot[:pr, c0 : c0 + cw]
                    if cp % 2 == 1:
                        nc.scalar.copy(out=dst, in_=ps[:pr, :cw])
                    else:
                        nc.vector.tensor_copy(out=dst, in_=ps[:pr, :cw])
                    if r == 0 and cp in (0, 1):
                        # prime the output-DMA stream before the tile finishes
                        nc.sync.dma_start(
                            out=out[0:pr, c0 : c0 + cw],
                            in_=ot[:pr, c0 : c0 + cw],
                        )
                if r == 0:
                    nc.sync.dma_start(
                        out=out[0:pr, 4 * COL_TILE :],
                        in_=ot[:pr, 4 * COL_TILE :],
                    )
                else:
                    nc.sync.dma_start(
                        out=out[r * ROW_TILE : r * ROW_TILE + pr, :], in_=ot[:pr, :]
                    )
    _install_multiwait_splitter(nc)
    return nc


def build_nc_k75() -> bass.Bass:
    """One matmul per output tile: stationary is the 25-row one-hot stacked
    three times along the contraction dim, the moving operand is the
    hi/mid/lo bf16 table planes stacked the same way. PSUM accumulates
    hi+mid+lo in fp32 in a single pass -> bit-exact f32 gather.

    Compute-engine SBUF accesses must start at a 32-aligned partition, so the
    three 25-row groups sit at partitions 0/32/64 (K=96). Pad partitions:
    one-hot rows compare labels against 25 (never matches -> 0), plane pad
    rows are zeroed via DMA so 0*0 keeps PSUM clean."""
    f32 = mybir.dt.float32
    bf16 = mybir.dt.bfloat16
    i32 = mybir.dt.int32
    GP = 32                  # partition stride between plane groups
    P3 = 3 * GP              # 96 = contraction dim incl. pads

    nc = bass.Bass()
    lbl = nc.dram_tensor("lbl", [1, ROWS], f32, kind="ExternalInput")
    proto = nc.dram_tensor("proto", [N_PROTO, D], f32, kind="ExternalInput")
    out = nc.dram_tensor("out", [ROWS, D], f32, kind="ExternalOutput")

    n_row_tiles = (ROWS + ROW_TILE - 1) // ROW_TILE
    n_col_tiles = (D + COL_TILE - 1) // COL_TILE

    with TileContext(nc) as tc:
        with (
            tc.tile_pool(name="const", bufs=1) as cpool,
            tc.tile_pool(name="psum", bufs=8, space="PSUM") as ppool,
            tc.tile_pool(name="outp", bufs=4) as opool,
        ):
            tbl75 = cpool.tile([P3, D], f32)
            lbl75 = cpool.tile([P3, ROWS], f32)
            for g in range(3):
                sl = slice(g * GP, g * GP + N_PROTO)
                nc.sync.dma_start(out=tbl75[sl, :], in_=proto[:])
                nc.sync.dma_start(
                    out=lbl75[g * GP : (g + 1) * GP, :],
                    in_=lbl[0].partition_broadcast(GP),
                )

            iota_i = cpool.tile([P3, 1], i32)
            nc.gpsimd.iota(iota_i, pattern=[[0, 1]], base=0, channel_multiplier=1)
            # group-local index, pads clamp to 25 which no label ever equals
            iota_q = cpool.tile([P3, 1], i32)
            nc.vector.tensor_scalar(
                out=iota_q, in0=iota_i, scalar1=GP - 1, scalar2=None,
                op0=mybir.AluOpType.bitwise_and,
            )
            iota_m = cpool.tile([P3, 1], i32)
            nc.vector.tensor_scalar(
                out=iota_m, in0=iota_q, scalar1=N_PROTO, scalar2=None,
                op0=mybir.AluOpType.min,
            )
            iota_f = cpool.tile([P3, 1], f32)
            nc.vector.tensor_copy(out=iota_f, in_=iota_m)

            oh = cpool.tile([P3, ROWS], bf16)
            nc.vector.tensor_scalar(
                out=oh, in0=lbl75, scalar1=iota_f[:, 0:1], scalar2=None,
                op0=mybir.AluOpType.is_equal,
            )

            # planes: partitions 0-24 hi, 32-56 mid, 64-88 lo (bf16, RN)
            planes = cpool.tile([P3, D], bf16)
            scrA = cpool.tile([P3, D], f32)
            scrB = cpool.tile([P3, D], f32)
            zpad = cpool.tile([GP - N_PROTO, D], bf16)
            nc.vector.memset(zpad, 0.0)
            for g in range(3):
                nc.sync.dma_start(
                    out=planes[g * GP + N_PROTO : (g + 1) * GP, :], in_=zpad
                )
            s0 = slice(0, N_PROTO)
            s1 = slice(GP, GP + N_PROTO)
            s2 = slice(2 * GP, 2 * GP + N_PROTO)
            # hi plane
            nc.vector.tensor_copy(out=planes[s0, :], in_=tbl75[s0, :])
            # mid plane: cast(x - f32(bf16(x)))
            nc.vector.tensor_copy(out=planes[s1, :], in_=tbl75[s1, :])
            nc.vector.tensor_copy(out=scrA[s1, :], in_=planes[s1, :])
            nc.vector.tensor_sub(out=planes[s1, :], in0=tbl75[s1, :], in1=scrA[s1, :])
            # lo plane: r1 = x - hi_f; mid = bf16(r1); lo = bf16(r1 - f32(mid))
            nc.vector.tensor_copy(out=planes[s2, :], in_=tbl75[s2, :])
            nc.vector.tensor_copy(out=scrA[s2, :], in_=planes[s2, :])
            nc.vector.tensor_sub(out=scrB[s2, :], in0=tbl75[s2, :], in1=scrA[s2, :])
            nc.vector.tensor_copy(out=planes[s2, :], in_=scrB[s2, :])
            nc.vector.tensor_copy(out=scrA[s2, :], in_=planes[s2, :])
            nc.vector.tensor_sub(out=planes[s2, :], in0=scrB[s2, :], in1=scrA[s2, :])

            for r in range(n_row_tiles):
                pr = min(ROW_TILE, ROWS - r * ROW_TILE)
                ot = opool.tile([ROW_TILE, D], f32)
                oh_sl = oh[:, r * ROW_TILE : r * ROW_TILE + pr]
                for c in range(n_col_tiles):
                    cn = min(COL_TILE, D - c * COL_TILE)
                    ps = ppool.tile([ROW_TILE, COL_TILE], f32)
                    nc.tensor.matmul(
                        ps[:pr, :cn],
                        oh_sl,
                        planes[:, c * COL_TILE : c * COL_TILE + cn],
                        start=True,
                        stop=True,
                    )
                    dst = ot[:pr, c * COL_TILE : c * COL_TILE + cn]
                    if c in (3, 7):
                        nc.scalar.copy(out=dst, in_=ps[:pr, :cn])
                    else:
                        nc.vector.tensor_copy(out=dst, in_=ps[:pr, :cn])
                nc.sync.dma_start(
                    out=out[r * ROW_TILE : r * ROW_TILE + pr, :], in_=ot[:pr, :]
                )
    _install_multiwait_splitter(nc)
    return nc


def build_nc(mode: str = _MODE) -> bass.Bass:
    if mode == "v4":
        return build_nc_v4()
    if mode == "v3":
        return build_nc_v3()
    if mode == "v2":
        return build_nc_v2()
    if mode == "k75":
        return build_nc_k75()
    f32 = mybir.dt.float32
    bf16 = mybir.dt.bfloat16

    nc = bass.Bass()
    lbl = nc.dram_tensor("lbl", [1, ROWS], f32, kind="ExternalInput")
    proto = nc.dram_tensor("proto", [N_PROTO, D], f32, kind="ExternalInput")
    out = nc.dram_tensor("out", [ROWS, D], f32, kind="ExternalOutput")

    n_row_tiles = (ROWS + ROW_TILE - 1) // ROW_TILE
    n_col_tiles = (D + COL_TILE - 1) // COL_TILE

    with TileContext(nc) as tc:
        with (
            tc.tile_pool(name="const", bufs=1) as cpool,
            tc.tile_pool(name="psum", bufs=8, space="PSUM") as ppool,
            tc.tile_pool(name="outp", bufs=4) as opool,
        ):
            tbl = cpool.tile([N_PROTO, D], f32)
            nc.sync.dma_start(out=tbl, in_=proto[:])

            lblb = cpool.tile([N_PROTO, ROWS], f32)
            nc.sync.dma_start(out=lblb, in_=lbl[0].partition_broadcast(N_PROTO))

            iot = cpool.tile([N_PROTO, 1], f32)
            nc.gpsimd.iota(
                iot,
                pattern=[[0, 1]],
                base=0,
                channel_multiplier=1,
                allow_small_or_imprecise_dtypes=True,
            )

            oh_dt = f32 if mode in ("f32", "f32r") else bf16
            oh = cpool.tile([N_PROTO, ROWS], oh_dt)
            nc.vector.tensor_scalar(
                out=oh,
                in0=lblb,
                scalar1=iot[:, 0:1],
                scalar2=None,
                op0=mybir.AluOpType.is_equal,
            )

            if mode in ("f32", "f32r"):
                planes = [tbl]
            else:
                # Exact f32 = hi + mid + lo, each bf16 (RN cast at each step).
                hi = cpool.tile([N_PROTO, D], bf16)
                nc.vector.tensor_copy(out=hi, in_=tbl)
                hi_f = cpool.tile([N_PROTO, D], f32)
                nc.vector.tensor_copy(out=hi_f, in_=hi)
                r1 = cpool.tile([N_PROTO, D], f32)
                nc.vector.tensor_sub(out=r1, in0=tbl, in1=hi_f)
                mid = cpool.tile([N_PROTO, D], bf16)
                nc.vector.tensor_copy(out=mid, in_=r1)
                planes = [hi, mid]
                if mode == "bf16x3":
                    mid_f = cpool.tile([N_PROTO, D], f32)
                    nc.vector.tensor_copy(out=mid_f, in_=mid)
                    r2 = cpool.tile([N_PROTO, D], f32)
                    nc.vector.tensor_sub(out=r2, in0=r1, in1=mid_f)
                    lo = cpool.tile([N_PROTO, D], bf16)
                    nc.vector.tensor_copy(out=lo, in_=r2)
                    planes.append(lo)

            for r in range(n_row_tiles):
                pr = min(ROW_TILE, ROWS - r * ROW_TILE)
                ot = opool.tile([ROW_TILE, D], f32)
                oh_sl = oh[:, r * ROW_TILE : r * ROW_TILE + pr]
                if mode == "f32r":
                    oh_sl = oh_sl.bitcast(mybir.dt.float32r)
                for c in range(n_col_tiles):
                    cn = min(COL_TILE, D - c * COL_TILE)
                    ps = ppool.tile([ROW_TILE, COL_TILE], f32)
                    for pi, plane in enumerate(planes):
                        rhs = plane[:, c * COL_TILE : c * COL_TILE + cn]
                        if mode == "f32r":
                            rhs = rhs.bitcast(mybir.dt.float32r)
                        nc.tensor.matmul(
                            ps[:pr, :cn],
                            oh_sl,
                            rhs,
                            start=(pi == 0),
                            stop=(pi == len(planes) - 1),
                        )
                    nc.vector.tensor_copy(
                        out=ot[:pr, c * COL_TILE : c * COL_TILE + cn],
                        in_=ps[:pr, :cn],
                    )
                nc.sync.dma_start(
                    out=out[r * ROW_TILE : r * ROW_TILE + pr, :], in_=ot[:pr, :]
                )
    _install_multiwait_splitter(nc)
    return nc


_NC_CACHE: dict[str, bass.Bass] = {}


def _get_nc(mode: str) -> bass.Bass:
    if mode not in _NC_CACHE:
        _NC_CACHE[mode] = build_nc(mode)
    return _NC_CACHE[mode]


def run(inputs, labels, prototypes, mode: str = _MODE, **spmd_kwargs):
    """Run the kernel; returns (output, BassKernelResults)."""
    lbl = np.asarray(labels).reshape(B, L)
    proto = np.ascontiguousarray(
        np.asarray(prototypes, dtype=np.float32).reshape(N_PROTO, D)
    )
    in_maps = []
    if mode in ("v3", "v4"):
        if mode == "v3":
            if V3_DENSE:
                planes = host_split_planes_dense(proto)
                mk = host_onehot_dense
            else:
                planes = host_split_planes(proto)
                mk = host_onehot_planes
        else:
            planes = host_split_planes(proto)
            mk = host_onehot_v4
        for c in range(N_CORES):
            lr = lbl[c * B_PER_CORE : (c + 1) * B_PER_CORE].reshape(ROWS)
            in_maps.append({"oh": mk(lr), "planes": planes})
    else:
        if mode == "v2":
            import ml_dtypes

            table_input = {"planes": host_split_planes(proto)}
            lbl_dt = ml_dtypes.bfloat16
        else:
            table_input = {"proto": proto}
            lbl_dt = np.float32
        for c in range(N_CORES):
            lf = (
                lbl[c * B_PER_CORE : (c + 1) * B_PER_CORE]
                .reshape(1, ROWS)
                .astype(lbl_dt)
            )
            in_maps.append({"lbl": lf, **table_input})
    res = run_bass_kernel_spmd(
        _get_nc(mode), in_maps, core_ids=list(range(N_CORES)), **spmd_kwargs
    )
    outs = [
        r["out"].reshape(B_PER_CORE, L, NCHAN, T, F) for r in res.results
    ]
    return np.concatenate(outs, axis=0), res


def kernel(inputs, labels, prototypes):
    out, _ = run(inputs, labels, prototypes)
    return out

